# revision 4
# baseline (speedup 1.0000x reference)
"""GNN segment-product kernel v2 for 8 Trainium2 NeuronCores.

Computation:
    h = x @ W                                  [N, 64]
    prod[d] = product of h[src[e]] over incoming edges of d (1 if none)
    neigh = where(deg > 0, prod, 0)
    out = neigh @ V.T                          [N, 256]

Design: the per-edge gather runs as GPSIMD ap_gather (d=4 bf16) from an
SBUF-resident transposed h table.  The table is split into 8 chunks of
~12.5k nodes; chunk ch lives on partitions 16ch..16ch+15, partition r
holding features 4r..4r+3 (d=4 pack, 16x4=64 features).  ap_gather index
streams are per-16-partition GPSIMD core == per chunk, so one gathered
column (one ~28ns index per core) carries 8 edges, one per chunk.  A greedy
host-side balancer assigns nodes to chunks so each dst's in-edges spread
evenly (K = max per-chunk count stays near deg/8).  DVE bf16 multiply trees
fold the K slots; 3 rounds of partition-shift DMA + bf16 muls combine the 8
chunk partials; PE applies V^T (4 accumulating rank-quarter matmuls per
128-dst tile); bf16 outputs, host upcasts and inverse-permutes.
"""

import math
import numpy as np
from contextlib import ExitStack

import concourse.bass as bass
import concourse.bacc as bacc
import concourse.mybir as mybir
import concourse.tile as tile
from concourse import bass_utils

P = 128
NCORES = 8
NCH = 8           # table chunks == gpsimd cores
GB = 256          # dsts per fold group
NI_MAX = 2560     # max gathered columns per ap_gather instruction
XT = 512          # phase-1 matmul tile columns
SGG = 2           # fold groups per epilogue super-group


def _host_prep(x, W, V, src, dst):
    N, F = x.shape
    R = W.shape[1]
    H = V.shape[0]
    npc = N // NCORES

    deg = np.bincount(dst, minlength=N)

    eorder = np.argsort(dst, kind="stable")
    src_by_dst = src[eorder]
    dstart = np.zeros(N + 1, np.int64)
    np.cumsum(deg, out=dstart[1:])

    sorder = np.argsort(src, kind="stable")
    dst_by_src = dst[sorder]
    odeg = np.bincount(src, minlength=N)
    sstart = np.zeros(N + 1, np.int64)
    np.cumsum(odeg, out=sstart[1:])

    # ---- greedy chunk assignment: balance each dst's edges over 8 chunks ----
    cnt = np.zeros((N, NCH), np.int16)
    qa = np.zeros(N, np.int8)
    tgt = (deg / NCH).astype(np.float32)
    order = np.argsort(-odeg, kind="stable")
    for n in order:
        ds = dst_by_src[sstart[n]:sstart[n + 1]]
        if len(ds) == 0:
            qa[n] = n % NCH
            continue
        c = cnt[ds].astype(np.float32) - tgt[ds][:, None]
        cost = np.maximum(c + 1.0, 0.0).sum(axis=0)
        q = int(np.argmin(cost))
        qa[n] = q
        cnt[ds, q] += 1
    for _pass in range(3):  # refinement passes
        for n in order:
            ds = dst_by_src[sstart[n]:sstart[n + 1]]
            if len(ds) == 0:
                continue
            cnt[ds, qa[n]] -= 1
            c = cnt[ds].astype(np.float32) - tgt[ds][:, None]
            cost = np.maximum(c + 1.0, 0.0).sum(axis=0)
            q = int(np.argmin(cost))
            qa[n] = q
            cnt[ds, q] += 1

    maxq = cnt.max(axis=1).astype(np.int32)

    # ---- dst ordering: sort by maxq, deal round-robin to cores ----
    dorder = np.argsort(maxq, kind="stable")
    nslots = GB * math.ceil(npc / GB)
    ngroups = nslots // GB
    K = np.zeros(ngroups, np.int64)
    for g in range(ngroups):
        lo, hi = g * GB * NCORES, min((g + 1) * GB * NCORES, N)
        K[g] = max(int(maxq[dorder[lo:hi]].max()) if lo < N else 1, 1)
    TOT = int((GB * K).sum())
    assert TOT % 16 == 0

    owner = np.empty(N, np.int64)
    ranks = np.arange(N)
    owner[dorder] = ranks % NCORES

    percore_nodes = []
    for c in range(NCORES):
        mine = np.where(owner == c)[0]
        po = mine[np.lexsort((mine, qa[mine]))]
        percore_nodes.append(po)

    # per (ch, c) widths / bases; cols 0,1 of each chunk reserved (zeros, ones)
    wq = np.zeros((NCH, NCORES), np.int64)
    for c in range(NCORES):
        for q in range(NCH):
            wq[q, c] = int((qa[percore_nodes[c]] == q).sum())
    base = np.zeros((NCH, NCORES), np.int64)
    NE = 0
    for q in range(NCH):
        acc = 2
        for c in range(NCORES):
            base[q, c] = acc
            acc += wq[q, c]
        NE = max(NE, acc)
    assert NE <= 32768, NE

    col = np.empty(N, np.int64)
    for c in range(NCORES):
        po = percore_nodes[c]
        for q in range(NCH):
            sel = po[qa[po] == q]
            col[sel] = base[q, c] + np.arange(len(sel))

    # ---- phase-1 x^T per core (fp16) ----
    ntile = math.ceil(npc / XT)
    XCOLS = ntile * XT
    xt_arrs = []
    for c in range(NCORES):
        po = percore_nodes[c]
        xs = np.zeros((F, XCOLS), np.float16)
        xs[:, :npc] = x[po].astype(np.float16).T
        xt_arrs.append(np.ascontiguousarray(xs))

    # W feature-quad split: w_pm[:, b, j, r] = W[b*128+k, 4r+j]
    w_pm = np.zeros((P, 2, 4, R // 4), np.float16)
    for b in range(2):
        for j in range(4):
            w_pm[:, b, j, :] = W[b * P:(b + 1) * P, j::4].astype(np.float16)
    # V^T quads: vt[r, j, :] = V.T[4r+j] = V[:, 4r+j]
    import ml_dtypes
    vt_q = np.zeros((R // 4, 4, H), ml_dtypes.bfloat16)
    for r in range(R // 4):
        for j in range(4):
            vt_q[r, j, :] = V[:, 4 * r + j].astype(ml_dtypes.bfloat16)

    # ---- gather index planes per core ----
    goff = np.zeros(ngroups + 1, np.int64)
    np.cumsum(GB * K, out=goff[1:])
    idx_arrs = []
    for c in range(NCORES):
        vals = np.zeros((NCH, TOT), np.int16)
        for g in range(ngroups):
            Kg = int(K[g])
            for j in range(GB):
                s = g * GB + j
                jj0 = goff[g] + j * Kg
                if s >= npc:
                    vals[:, jj0:jj0 + Kg] = 1
                    continue
                n = dorder[s * NCORES + c]
                if deg[n] == 0:
                    vals[:, jj0:jj0 + Kg] = 0
                    continue
                vals[:, jj0:jj0 + Kg] = 1
                ss = src_by_dst[dstart[n]:dstart[n + 1]]
                qs = qa[ss]
                cs = col[ss]
                for q in range(NCH):
                    cq = cs[qs == q]
                    vals[q, jj0:jj0 + len(cq)] = cq.astype(np.int16)
        plane = np.zeros((P, TOT // 16), np.int16)
        v3 = vals.reshape(NCH, TOT // 16, 16)
        for q in range(NCH):
            plane[16 * q:16 * (q + 1), :] = v3[q].T
        idx_arrs.append(plane)

    # gather instruction batches (merge groups)
    batches = []
    cur, cur_cols = [], 0
    for g in range(ngroups):
        c_g = GB * int(K[g])
        if cur and cur_cols + c_g > NI_MAX:
            batches.append(cur)
            cur, cur_cols = [], 0
        cur.append(g)
        cur_cols += c_g
    if cur:
        batches.append(cur)

    meta = dict(
        N=N, F=F, R=R, H=H, npc=npc,
        NE=int(NE), TOT=TOT, ngroups=ngroups, K=[int(k) for k in K],
        goff=[int(o) for o in goff], batches=batches,
        nslots=nslots, ntile=ntile, XCOLS=XCOLS,
        wq=wq.tolist(), qbase=base.tolist(),
    )
    return meta, percore_nodes, dorder, idx_arrs, xt_arrs, w_pm, vt_q


def _build_program(meta):
    F = meta["F"]; R = meta["R"]; H = meta["H"]
    NE = meta["NE"]; TOT = meta["TOT"]
    ngroups = meta["ngroups"]; K = meta["K"]; goff = meta["goff"]
    batches = meta["batches"]; nslots = meta["nslots"]
    ntile = meta["ntile"]; XCOLS = meta["XCOLS"]
    wq = meta["wq"]; qbase = meta["qbase"]
    f32 = mybir.dt.float32
    f16 = mybir.dt.float16
    bf16 = mybir.dt.bfloat16
    i16 = mybir.dt.int16

    NIB = max(sum(GB * K[g] for g in b) for b in batches)

    nc = bacc.Bacc(
        "TRN2", target_bir_lowering=False, debug=False,
        enable_asserts=False, num_devices=NCORES,
    )
    x_t = nc.dram_tensor("x_t", [F, XCOLS], f16, kind="ExternalInput")
    w_pm = nc.dram_tensor("w_pm", [P, 2 * R], f16, kind="ExternalInput")
    vt_q = nc.dram_tensor("vt_q", [R // 4, 4 * H], bf16, kind="ExternalInput")
    idx = nc.dram_tensor("idx", [P, TOT // 16], i16, kind="ExternalInput")
    out = nc.dram_tensor("out", [nslots, H], f32, kind="ExternalOutput")

    with tile.TileContext(nc) as tc:
        with ExitStack() as ctx:
            dram = ctx.enter_context(tc.tile_pool(name="dram", bufs=1, space="DRAM"))
            sb = ctx.enter_context(tc.tile_pool(name="sb", bufs=1))

            shard = dram.tile([16, 4 * XCOLS], bf16)
            ag = dram.tile([NCORES * 16, 4 * XCOLS], bf16, addr_space="Shared")

            tbl = sb.tile([P, NE, 4], bf16)
            ixt = sb.tile([P, TOT // 16], i16)
            nc.sync.dma_start(out=ixt[:], in_=idx[:, :])
            w_sb = sb.tile([P, 2, 4, R // 4], f16)
            nc.sync.dma_start(
                out=w_sb[:],
                in_=w_pm[:, :].rearrange("p (b j r) -> p b j r", b=2, j=4))
            v_sb = sb.tile([R // 4, 4, H], bf16)
            nc.sync.dma_start(out=v_sb[:], in_=vt_q[:, :].rearrange("p (b h) -> p b h", b=4))

            # ---- phase 1: h^T quads, streamed to DRAM shard ----
            with tc.tile_pool(name="ph1", bufs=3) as ph1, \
                 tc.tile_pool(name="ps1", bufs=2, space="PSUM") as ps1:
                for t in range(ntile):
                    c0 = t * XT
                    xt_b = ph1.tile([P, 2, XT], f16, tag="xt")
                    [nc.sync, nc.gpsimd][t % 2].dma_start(
                        out=xt_b[:],
                        in_=x_t[:, c0:c0 + XT].rearrange("(b p) n -> p b n", p=P),
                    )
                    hp = ps1.tile([96, XT], f32, tag="hp")
                    hp2 = ps1.tile([16, XT], f32, tag="hp2")
                    for j in range(4):
                        dst = hp[32 * j:32 * j + 16, :] if j < 3 else hp2[:, :]
                        for b in range(2):
                            nc.tensor.matmul(
                                out=dst,
                                lhsT=w_sb[:, b, j, :],
                                rhs=xt_b[:, b, :],
                                start=(b == 0),
                                stop=(b == 1),
                            )
                    pk = ph1.tile([16, XT, 4], bf16, tag="pk")
                    for j in range(3):
                        nc.scalar.copy(out=pk[:, :, j], in_=hp[32 * j:32 * j + 16, :])
                    nc.scalar.copy(out=pk[:, :, 3], in_=hp2[:, :])
                    nc.scalar.dma_start(
                        out=shard[:, 4 * c0:4 * (c0 + XT)],
                        in_=pk[:].rearrange("p a b -> p (a b)"),
                    )
            nc.gpsimd.collective_compute(
                "AllGather",
                mybir.AluOpType.bypass,
                replica_groups=[list(range(NCORES))],
                ins=[shard[:].opt()],
                outs=[ag[:].opt()],
            )
            # ---- table load + reserved cols (same col ids in every chunk) ----
            nc.vector.memset(tbl[:, 0, :], 0.0)
            nc.vector.memset(tbl[:, 1, :], 1.0)
            ag3 = ag[:, :].rearrange("p (a b) -> p a b", b=4)
            _qeng = [nc.sync, nc.scalar, nc.gpsimd]
            _qi = 0
            for c in range(NCORES):
                coff = 0
                for q in range(NCH):
                    w = wq[q][c]
                    if w == 0:
                        continue
                    b0 = qbase[q][c]
                    _qeng[_qi % 3].dma_start(
                        out=tbl[16 * q:16 * (q + 1), b0:b0 + w, :],
                        in_=ag3[16 * c:16 * (c + 1), coff:coff + w, :],
                    )
                    _qi += 1
                    coff += w

            # ---- phase 2 ----
            with tc.tile_pool(name="gth", bufs=3) as gth, \
                 tc.tile_pool(name="stg", bufs=3) as stg, \
                 tc.tile_pool(name="tmp", bufs=2) as tmpp, \
                 tc.tile_pool(name="ops", bufs=3) as ops, \
                 tc.tile_pool(name="pso", bufs=2, space="PSUM") as pso:

                def fold_group(g, gt, cbase, sg_tile):
                    Kg = K[g]
                    o = goff[g] - cbase
                    b3 = gt[:, o:o + GB * Kg, :].rearrange(
                        "p (j k) c -> p j k c", k=Kg)
                    m = Kg
                    while m > 1:
                        if m % 2:
                            nc.vector.tensor_mul(
                                out=b3[:, :, 0, :], in0=b3[:, :, 0, :],
                                in1=b3[:, :, m - 1, :],
                            )
                            m -= 1
                            if m == 1:
                                break
                        half = m // 2
                        nc.vector.tensor_mul(
                            out=b3[:, :, 0:half, :],
                            in0=b3[:, :, 0:half, :],
                            in1=b3[:, :, half:m, :],
                        )
                        m = half
                    nc.vector.tensor_copy(
                        out=sg_tile[:, (g % SGG) * GB:(g % SGG + 1) * GB, :],
                        in_=b3[:, :, 0, :] if Kg > 1 else b3[:, :, 0, :],
                    )

                def epilogue(sg, sg_tile, ncols):
                    # combine the 8 chunk bands: 3 halving rounds
                    t1 = tmpp.tile([P, SGG * GB, 4], bf16, tag="t1")
                    step = 1
                    nb = NCH
                    while nb > 1:
                        for i in range(nb // 2):
                            srcb = (2 * i + 1) * step * 16
                            dstb = (2 * i) * step * 16
                            nc.sync.dma_start(
                                out=t1[dstb:dstb + 16, :ncols, :],
                                in_=sg_tile[srcb:srcb + 16, :ncols, :])
                        for i in range(nb // 2):
                            dstb = (2 * i) * step * 16
                            nc.vector.tensor_mul(
                                out=sg_tile[dstb:dstb + 16, :ncols, :],
                                in0=sg_tile[dstb:dstb + 16, :ncols, :],
                                in1=t1[dstb:dstb + 16, :ncols, :])
                        nb //= 2
                        step *= 2
                    for tt in range(ncols // P):
                        dlo = tt * P
                        o_ps = pso.tile([P, H], f32, tag="o_ps")
                        for j in range(4):
                            nc.tensor.matmul(
                                out=o_ps[:],
                                lhsT=sg_tile[0:16, dlo:dlo + P, j],
                                rhs=v_sb[:, j, :],
                                start=(j == 0),
                                stop=(j == 3),
                            )
                        o_sb = ops.tile([P, H], f32, tag="o_sb")
                        nc.scalar.copy(out=o_sb[:], in_=o_ps[:])
                        [nc.sync, nc.scalar][tt % 2].dma_start(
                            out=out[sg * SGG * GB + dlo:sg * SGG * GB + dlo + P, :],
                            in_=o_sb[:],
                        )

                open_sg = {}
                for bgroups in batches:
                    gt = gth.tile([P, NIB, 4], bf16, tag="gt")
                    cbase = goff[bgroups[0]]
                    cols = sum(GB * K[g] for g in bgroups)
                    nc.gpsimd.ap_gather(
                        out_ap=gt[:, :cols, :], in_ap=tbl[:],
                        idxs_ap=ixt[:, cbase // 16:(cbase + cols) // 16],
                        channels=P, num_elems=NE, d=4, num_idxs=cols,
                    )
                    for g in bgroups:
                        sg = g // SGG
                        if sg not in open_sg:
                            open_sg[sg] = stg.tile([P, SGG * GB, 4], bf16,
                                                   tag="sg", name=f"sg{sg}")
                        fold_group(g, gt, cbase, open_sg[sg])
                        if g == min(ngroups, (sg + 1) * SGG) - 1:
                            ncols = (min(ngroups, (sg + 1) * SGG) - sg * SGG) * GB
                            epilogue(sg, open_sg.pop(sg), ncols)
    nc.compile()
    return nc


def kernel(x, W, V, src, dst):
    import ml_dtypes
    x = np.asarray(x); W = np.asarray(W); V = np.asarray(V)
    src = np.asarray(src); dst = np.asarray(dst)
    meta, percore_nodes, dorder, idx_arrs, xt_arrs, w_pm, vt_q = _host_prep(
        x, W, V, src, dst)
    nc = _build_program(meta)
    in_maps = [
        {"x_t": xt_arrs[c],
         "w_pm": np.ascontiguousarray(w_pm.reshape(P, -1)),
         "vt_q": np.ascontiguousarray(vt_q.reshape(meta["R"] // 4, -1)).view(np.int16),
         "idx": idx_arrs[c]}
        for c in range(NCORES)
    ]
    res = bass_utils.run_bass_kernel_spmd(nc, in_maps, core_ids=list(range(NCORES)))
    N, H, npc = meta["N"], meta["H"], meta["npc"]
    out_full = np.empty((N, H), dtype=np.float32)
    for c in range(NCORES):
        o = res.results[c]["out"]
        if o.dtype != np.float32:
            o = o.view(ml_dtypes.bfloat16).astype(np.float32) if o.dtype == np.int16 else o.astype(np.float32)
        nodes = dorder[np.arange(npc) * NCORES + c]
        out_full[nodes] = o[:npc]
    return out_full
